# revision 8
# baseline (speedup 1.0000x reference)
"""MemoryBank.get_all_distances Trainium2 kernel.

emb_batch [64, 64] f32, bank [131072, 64] f32 -> distances [64, 131072] f32
  distances[n, b] = || bank[b] - emb[n] ||_2

Strategy: shard bank rows across 8 cores (16384 rows each). On the host we
only re-layout (transpose + stack) and encode the shard as fp8e4m3 with a
2^2 block scale folded into the encoding; all arithmetic runs on device:

  dist^2[n, b] = ||e_n||^2 + (-e_n/2) . (4 b_b) + (1/16) (4 b_b)^2

Per core the shard is fed as bt [128, 8192] fp8 (= 4*bank values):
partitions 0-63 hold dim d of bank columns 0..8191 of the shard, partitions
64-127 of columns 8192..16383. Each 512-column block is one fp8 DoubleRow
matmul whose two k-tiles fuse the dot product and the row-norm term:

  psum[:, blk] = W[:,0].T @ X[:,0,blk]  +  W[:,1].T @ X[:,1,blk]
     W[:,0] = blockdiag(-embT/2),  X[:,0] = 4*bt   (dot term)
     W[:,1] = blockdiag(1/16),     X[:,1] = (4*bt)^2 (squares, DVE/GpSimd)

The ACT engine finishes with sqrt(psum + bias) -> bf16, bias[n] = ||e_n||^2
(f32 via DVE square + free-dim reduce). The host widens bf16 -> f32.

Scheduling notes (from perfetto traces):
 - every dma_start costs ~0.65us on its issuing engine; each HWDGE ring is
   FIFO at ~240 GB/s -> inputs split across both HWDGE rings, all issued up
   front; outputs go per-1024 to gpsimd SWDGE (evens) / sync ring (odds).
 - ewt is first on the sync ring: the bias reduce and ACT-table warm-up
   must not wait behind the bank stream (v2 lost ~5us to that).
 - the PE boots HAM-throttled at half clock; 4 dummy matmuls during the
   DMA-latency window warm it up.
 - the Sqrt ACT table (~1.3us) is preloaded by a dummy activation with the
   same bias-AP signature as the real ones.
"""

import numpy as np

BANK = 131072
DIM = 64
BATCH = 64
N_CORES = 8
SHARD = BANK // N_CORES  # 16384 bank rows per core
HALF = SHARD // 2  # 8192 columns per partition-half
W = 2048  # chunk width (bank columns per partition-half)
NBLK = 512  # psum bank / matmul block
ABLK = 1024  # activation / output-DMA block
N_WARM = 4  # PE warm-up dummy matmuls

_cache = {}

# test.py reads this after calling kernel() to get profiling info.
last_run = None


def _build(half=HALF, w=W):
    import concourse.mybir as mybir
    import concourse.tile as tile
    from concourse import bacc

    f32 = mybir.dt.float32
    bf16 = mybir.dt.bfloat16
    fp8 = mybir.dt.float8e4
    SQRT = mybir.ActivationFunctionType.Sqrt
    DR = mybir.MatmulPerfMode.DoubleRow
    nchunk = half // w
    nblk = w // NBLK

    nc = bacc.Bacc(
        "TRN2", target_bir_lowering=False, debug=False, num_devices=N_CORES
    )
    bt = nc.dram_tensor("bt", [128, half], fp8, kind="ExternalInput").ap()
    ew = nc.dram_tensor("ew", [128, 128], f32, kind="ExternalInput").ap()
    ewt = nc.dram_tensor("ewt", [128, DIM], f32, kind="ExternalInput").ap()
    o = nc.dram_tensor("o", [128, half], bf16, kind="ExternalOutput").ap()

    with tile.TileContext(nc) as tc:
        with (
            tc.tile_pool(name="singles", bufs=1) as singles,
            tc.tile_pool(name="x_pool", bufs=nchunk) as x_pool,
            tc.tile_pool(name="out_pool", bufs=3) as out_pool,
            tc.tile_pool(name="psum", bufs=2, space="PSUM") as psum,
        ):
            # --- input streams, all doorbells rung up front ---------------
            ewt2 = singles.tile([128, DIM], f32)
            nc.sync.dma_start(out=ewt2, in_=ewt)
            ew2 = singles.tile([128, 128], f32)
            nc.scalar.dma_start(out=ew2, in_=ew)
            xs = []
            for ci in range(nchunk):
                xs.append(
                    x_pool.tile([128, nblk, 2, NBLK], fp8, name=f"x{ci}", tag="x")
                )
            for ci in range(0, nchunk, 2):
                nc.sync.dma_start(
                    out=xs[ci][:, :, 0, :], in_=bt[:, ci * w : (ci + 1) * w]
                )
            for ci in range(1, nchunk, 2):
                nc.scalar.dma_start(
                    out=xs[ci][:, :, 0, :], in_=bt[:, ci * w : (ci + 1) * w]
                )

            # --- PE warm-up against the HAM clock gate --------------------
            dummy_w = singles.tile([128, 128], fp8)
            nc.gpsimd.memset(dummy_w, 0.0)
            dummy_r = singles.tile([128, 512], fp8)
            nc.gpsimd.memset(dummy_r, 0.0)
            ps_warm = psum.tile([128, w], f32, tag="ps")
            for _ in range(N_WARM):
                nc.tensor.matmul(
                    ps_warm[:, 0:512], lhsT=dummy_w, rhs=dummy_r,
                    start=True, stop=True,
                )

            # Preload the Sqrt ACT table with the production signature
            # (bias as AP) so the real activations hit a warm table.
            zt = singles.tile([128, 1], f32)
            nc.vector.memset(zt, 0.0)
            warm = singles.tile([128, 1], f32)
            nc.scalar.activation(out=warm, in_=zt, func=SQRT, bias=zt, scale=1.0)

            # bias[m] = ||e_{m%64}||^2 via DVE square + free-dim reduce.
            sq_ewt = singles.tile([128, DIM], f32)
            nc.vector.tensor_mul(sq_ewt, ewt2, ewt2)
            bias = singles.tile([128, 1], f32)
            nc.vector.tensor_reduce(
                out=bias,
                in_=sq_ewt,
                axis=mybir.AxisListType.X,
                op=mybir.AluOpType.add,
            )

            # DoubleRow stationary [128, 2, 128] fp8:
            #   k-tile 0: blockdiag(-embT/2), k-tile 1: blockdiag(1/16).
            wdr_f = singles.tile([128, 2, 128], f32)
            nc.vector.memset(wdr_f, 0.0)
            nc.vector.tensor_scalar_mul(
                wdr_f[0:64, 0, 0:64], ew2[0:64, 0:DIM], -0.5
            )
            nc.vector.tensor_scalar_mul(
                wdr_f[64:128, 0, 64:128], ew2[64:128, 0:DIM], -0.5
            )
            nc.vector.memset(wdr_f[0:64, 1, 0:64], 0.0625)
            nc.vector.memset(wdr_f[64:128, 1, 64:128], 0.0625)
            wdr = singles.tile([128, 2, 128], fp8)
            nc.vector.tensor_copy(out=wdr, in_=wdr_f)

            # --- main pipeline --------------------------------------------
            for ci in range(nchunk):
                x = xs[ci]
                ps = psum.tile([128, w], f32, tag="ps")
                # squares into k-tile 1; split across DVE / GpSimd.
                sq_eng = nc.vector if ci % 2 == 0 else nc.gpsimd
                sq_eng.tensor_mul(x[:, :, 1, :], x[:, :, 0, :], x[:, :, 0, :])
                for j in range(nblk):
                    nc.tensor.matmul(
                        ps[:, j * NBLK : (j + 1) * NBLK],
                        lhsT=wdr,
                        rhs=x[:, j],
                        start=True,
                        stop=True,
                        perf_mode=DR,
                    )
                for j in range(w // ABLK):
                    sl = slice(j * ABLK, (j + 1) * ABLK)
                    gs = slice(ci * w + j * ABLK, ci * w + (j + 1) * ABLK)
                    out_c = out_pool.tile([128, ABLK], bf16)
                    nc.scalar.activation(
                        out=out_c, in_=ps[:, sl], func=SQRT, bias=bias,
                        scale=1.0,
                    )
                    if (ci * (w // ABLK) + j) % 2 == 0:
                        nc.gpsimd.dma_start(out=o[:, gs], in_=out_c)
                    else:
                        nc.sync.dma_start(out=o[:, gs], in_=out_c)

    nc.compile()
    return nc


def _get_nc():
    if "nc" not in _cache:
        _cache["nc"] = _build()
    return _cache["nc"]


def _prep_inputs(emb_batch, bank):
    """Host-side re-layout (shard, transpose, stack) + fp8 encode.

    The 2^2 scale is part of the fp8 encoding (pure exponent shift) so the
    squared values stay in e4m3 normal range; the device multiplies the
    squares' k-tile by 1/16.
    """
    import ml_dtypes

    fp8 = ml_dtypes.float8_e4m3
    emb_batch = np.asarray(emb_batch, dtype=np.float32)
    bank = np.asarray(bank, dtype=np.float32)
    # [128, 128]: rows 0-63 and 64-127 both embT; cols duplicated.
    ew_host = np.ascontiguousarray(np.tile(emb_batch.T, (2, 2)))
    # [128, 64] query-major copy for the on-device ||e||^2 reduce.
    ewt_host = np.ascontiguousarray(np.tile(emb_batch, (2, 1)))
    bankT = bank.T * 4.0  # [64, BANK] scaled view
    in_maps = []
    for c in range(N_CORES):
        sh = bankT[:, c * SHARD : (c + 1) * SHARD]
        btc = np.ascontiguousarray(
            np.concatenate([sh[:, :HALF], sh[:, HALF:]], axis=0)
        ).astype(fp8)
        in_maps.append({"bt": btc, "ew": ew_host, "ewt": ewt_host})
    return in_maps


def kernel(emb_batch, bank):
    global last_run
    from concourse.bass_utils import run_bass_kernel_spmd

    nc = _get_nc()
    in_maps = _prep_inputs(emb_batch, bank)
    res = run_bass_kernel_spmd(nc, in_maps, core_ids=list(range(N_CORES)))
    last_run = res
    out = np.empty((BATCH, BANK), dtype=np.float32)
    for c in range(N_CORES):
        oc = res.results[c]["o"]  # [128, HALF] bf16: rows (h*64 + n)
        oc = np.asarray(oc).astype(np.float32)
        out[:, c * SHARD : c * SHARD + HALF] = oc[0:64]
        out[:, c * SHARD + HALF : (c + 1) * SHARD] = oc[64:128]
    return out


# revision 9
# speedup vs baseline: 1.2173x; 1.2173x over previous
"""MemoryBank.get_all_distances Trainium2 kernel.

emb_batch [64, 64] f32, bank [131072, 64] f32 -> distances [64, 131072] f32
  distances[n, b] = || bank[b] - emb[n] ||_2

Strategy: shard bank rows across 8 cores (16384 rows each). On the host we
only re-layout (transpose + stack + bf16 cast) the shard; all arithmetic
runs on device:

  dist^2[n, b] = (||e_n||^2 + ||b_b||^2) - 2 e_n . b_b

MemoryBank rows are L2-normalized (see MemoryBank._create), so ||b_b||^2
== 1 and the whole first term is the per-query constant ||e_n||^2 + 1,
computed on device (DVE square + reduce + add). kernel() verifies the
normalization on the host (read-only check) and falls back to a variant
that computes ||b_b||^2 on device (DVE squares + ones-matmul) if it does
not hold.

Per core the shard is fed as bt [128, 8192] bf16: partitions 0-63 hold dim
d of bank columns 0..8191 of the shard, partitions 64-127 of columns
8192..16383. Each 512-column block is one K=128/M=128 bf16 matmul with the
block-diagonal stationary [[-2*embT,0],[0,-2*embT]]; the ACT engine
finishes with sqrt(psum + bias) -> bf16 and the host widens to f32.

Scheduling notes (from perfetto traces):
 - every dma_start costs ~0.65us on its issuing engine; each HWDGE ring is
   FIFO at ~240 GB/s. Inputs are split 5/3 across the sync/scalar rings and
   all issued up front; outputs go per-1024 to gpsimd SWDGE (evens) / the
   by-then-idle sync ring (odds).
 - ewt rides first on the sync ring and the ACT-table warm-up sits right
   after ew on the scalar ring, before the input issues, so the Sqrt table
   (~2.6us) and bias are ready before the first real activation.
 - the PE boots HAM-throttled at half clock; 3 dummy matmuls during the
   DMA-latency window warm it up.
"""

import numpy as np

BANK = 131072
DIM = 64
BATCH = 64
N_CORES = 8
SHARD = BANK // N_CORES  # 16384 bank rows per core
HALF = SHARD // 2  # 8192 columns per partition-half
W = 1024  # chunk width (bank columns per partition-half)
NBLK = 512  # psum bank / matmul block
N_WARM = 3  # PE warm-up dummy matmuls
N_SYNC_IN = 5  # input chunks issued on the sync ring (rest on scalar)

_cache = {}

# test.py reads this after calling kernel() to get profiling info.
last_run = None


def _build(fused_norm=True, half=HALF, w=W):
    import concourse.mybir as mybir
    import concourse.tile as tile
    from concourse import bacc

    f32 = mybir.dt.float32
    bf16 = mybir.dt.bfloat16
    SQRT = mybir.ActivationFunctionType.Sqrt
    nchunk = half // w

    nc = bacc.Bacc(
        "TRN2", target_bir_lowering=False, debug=False, num_devices=N_CORES
    )
    bt = nc.dram_tensor("bt", [128, half], bf16, kind="ExternalInput").ap()
    ew = nc.dram_tensor("ew", [128, 128], f32, kind="ExternalInput").ap()
    ewt = nc.dram_tensor("ewt", [128, DIM], f32, kind="ExternalInput").ap()
    o = nc.dram_tensor("o", [128, half], bf16, kind="ExternalOutput").ap()

    with tile.TileContext(nc) as tc:
        with (
            tc.tile_pool(name="singles", bufs=1) as singles,
            tc.tile_pool(name="bt_pool", bufs=nchunk) as bt_pool,
            tc.tile_pool(name="sq_pool", bufs=3) as sq_pool,
            tc.tile_pool(name="out_pool", bufs=3) as out_pool,
            tc.tile_pool(name="psum", bufs=4, space="PSUM") as psum,
        ):
            # --- input streams, all doorbells rung up front ---------------
            # sync ring: ewt (tiny, feeds bias), then the first chunks.
            # scalar ring: ew (feeds the stationary), ACT warm-up, then the
            # remaining chunks (so table load isn't queued behind 5 DMAs).
            ewt2 = singles.tile([128, DIM], f32)
            nc.sync.dma_start(out=ewt2, in_=ewt)
            ew2 = singles.tile([128, 128], f32)
            nc.scalar.dma_start(out=ew2, in_=ew)

            # Preload the Sqrt ACT table with the production signature.
            zt = singles.tile([128, 1], f32)
            nc.vector.memset(zt, 0.0)
            warm = singles.tile([128, 1], f32)
            nc.scalar.activation(out=warm, in_=zt, func=SQRT, bias=zt, scale=1.0)

            bts = []
            for ci in range(nchunk):
                bts.append(
                    bt_pool.tile([128, w], bf16, name=f"bt{ci}", tag="bt")
                )
            for ci in range(nchunk):
                eng = nc.sync if ci < N_SYNC_IN else nc.scalar
                eng.dma_start(out=bts[ci], in_=bt[:, ci * w : (ci + 1) * w])

            # --- PE warm-up against the HAM clock gate --------------------
            dummy_w = singles.tile([128, 128], bf16)
            nc.gpsimd.memset(dummy_w, 0.0)
            dummy_r = singles.tile([128, 512], bf16)
            nc.gpsimd.memset(dummy_r, 0.0)
            ps_warm = psum.tile([128, w], f32, tag="ps")
            for _ in range(N_WARM):
                nc.tensor.matmul(
                    ps_warm[:, 0:512], lhsT=dummy_w, rhs=dummy_r,
                    start=True, stop=True,
                )

            # bias[m] = ||e_{m%64}||^2 (+1 for the normalized bank rows),
            # f32 via DVE square + free-dim reduce.
            sq_ewt = singles.tile([128, DIM], f32)
            nc.vector.tensor_mul(sq_ewt, ewt2, ewt2)
            bias = singles.tile([128, 1], f32)
            nc.vector.tensor_reduce(
                out=bias,
                in_=sq_ewt,
                axis=mybir.AxisListType.X,
                op=mybir.AluOpType.add,
            )
            if fused_norm:
                bias1 = singles.tile([128, 1], f32)
                nc.vector.tensor_scalar_add(bias1, bias, 1.0)
                bias = bias1

            # Block-diagonal stationaries [128, 128] bf16.
            em2bd_f = singles.tile([128, 128], f32)
            nc.vector.memset(em2bd_f, 0.0)
            nc.vector.tensor_scalar_mul(
                em2bd_f[0:64, 0:64], ew2[0:64, 0:DIM], -2.0
            )
            nc.vector.tensor_scalar_mul(
                em2bd_f[64:128, 64:128], ew2[64:128, 0:DIM], -2.0
            )
            em2bd = singles.tile([128, 128], bf16)
            nc.vector.tensor_copy(out=em2bd, in_=em2bd_f)

            if not fused_norm:
                onesbd_f = singles.tile([128, 128], f32)
                nc.vector.memset(onesbd_f, 0.0)
                nc.vector.memset(onesbd_f[0:64, 0:64], 1.0)
                nc.vector.memset(onesbd_f[64:128, 64:128], 1.0)
                onesbd = singles.tile([128, 128], bf16)
                nc.vector.tensor_copy(out=onesbd, in_=onesbd_f)

            # --- main pipeline --------------------------------------------
            for ci in range(nchunk):
                bt_c = bts[ci]
                ps = psum.tile([128, w], f32, tag="ps")
                for j in range(w // NBLK):
                    sl = slice(j * NBLK, (j + 1) * NBLK)
                    nc.tensor.matmul(
                        ps[:, sl], lhsT=em2bd, rhs=bt_c[:, sl],
                        start=True, stop=fused_norm,
                    )
                if not fused_norm:
                    sq_c = sq_pool.tile([128, w], bf16)
                    nc.vector.tensor_mul(sq_c, bt_c, bt_c)
                    for j in range(w // NBLK):
                        sl = slice(j * NBLK, (j + 1) * NBLK)
                        nc.tensor.matmul(
                            ps[:, sl], lhsT=onesbd, rhs=sq_c[:, sl],
                            start=False, stop=True,
                        )
                cs = slice(ci * w, (ci + 1) * w)
                out_c = out_pool.tile([128, w], bf16)
                nc.scalar.activation(
                    out=out_c, in_=ps, func=SQRT, bias=bias, scale=1.0
                )
                if ci % 2 == 0:
                    nc.gpsimd.dma_start(out=o[:, cs], in_=out_c)
                else:
                    nc.sync.dma_start(out=o[:, cs], in_=out_c)

    nc.compile()
    return nc


def _get_nc(fused_norm):
    key = ("nc", fused_norm)
    if key not in _cache:
        _cache[key] = _build(fused_norm)
    return _cache[key]


def _prep_inputs(emb_batch, bank):
    """Host-side re-layout only (shard, transpose, stack, bf16 cast)."""
    import ml_dtypes

    bf16 = ml_dtypes.bfloat16
    emb_batch = np.asarray(emb_batch, dtype=np.float32)
    bank = np.asarray(bank, dtype=np.float32)
    # [128, 128]: rows 0-63 and 64-127 both embT; cols duplicated.
    ew_host = np.ascontiguousarray(np.tile(emb_batch.T, (2, 2)))
    # [128, 64] query-major copy for the on-device ||e||^2 reduce.
    ewt_host = np.ascontiguousarray(np.tile(emb_batch, (2, 1)))
    bankT = bank.T  # [64, BANK] view
    in_maps = []
    for c in range(N_CORES):
        sh = bankT[:, c * SHARD : (c + 1) * SHARD]
        btc = np.ascontiguousarray(
            np.concatenate([sh[:, :HALF], sh[:, HALF:]], axis=0)
        ).astype(bf16)
        in_maps.append({"bt": btc, "ew": ew_host, "ewt": ewt_host})
    return in_maps


def kernel(emb_batch, bank):
    global last_run
    from concourse.bass_utils import run_bass_kernel_spmd

    bank = np.asarray(bank, dtype=np.float32)
    # Read-only validation: MemoryBank rows are L2-normalized. Use the
    # fused-constant kernel when that holds, the general one otherwise.
    norms = np.einsum("bd,bd->b", bank, bank)
    fused_norm = bool(np.abs(norms - 1.0).max() < 1e-3)

    nc = _get_nc(fused_norm)
    in_maps = _prep_inputs(emb_batch, bank)
    res = run_bass_kernel_spmd(nc, in_maps, core_ids=list(range(N_CORES)))
    last_run = res
    out = np.empty((BATCH, BANK), dtype=np.float32)
    for c in range(N_CORES):
        oc = res.results[c]["o"]  # [128, HALF] bf16: rows (h*64 + n)
        oc = np.asarray(oc).astype(np.float32)
        out[:, c * SHARD : c * SHARD + HALF] = oc[0:64]
        out[:, c * SHARD + HALF : (c + 1) * SHARD] = oc[64:128]
    return out


# revision 10
# speedup vs baseline: 1.2666x; 1.0405x over previous
"""MemoryBank.get_all_distances Trainium2 kernel.

emb_batch [64, 64] f32, bank [131072, 64] f32 -> distances [64, 131072] f32
  distances[n, b] = || bank[b] - emb[n] ||_2

Strategy: shard bank rows across 8 cores (16384 rows each). On the host we
only re-layout (transpose + stack + bf16 cast) the shard; all arithmetic
runs on device:

  dist^2[n, b] = (||e_n||^2 + ||b_b||^2) - 2 e_n . b_b

MemoryBank rows are L2-normalized (see MemoryBank._create), so ||b_b||^2
== 1 and the whole first term is the per-query constant ||e_n||^2 + 1,
computed on device (DVE square + reduce + add). kernel() verifies the
normalization on the host (read-only check) and falls back to a variant
that computes ||b_b||^2 on device (DVE squares + ones-matmul) if it does
not hold.

Per core the shard is fed as bt [128, 8192] bf16: partitions 0-63 hold dim
d of bank columns 0..8191 of the shard, partitions 64-127 of columns
8192..16383. Each 512-column block is one K=128/M=128 bf16 matmul with the
block-diagonal stationary [[-2*embT,0],[0,-2*embT]]; the ACT engine
finishes with sqrt(psum + bias) -> bf16 and the host widens to f32.

Scheduling notes (from perfetto traces):
 - every dma_start costs ~0.65us on its issuing engine; each HWDGE ring is
   FIFO at ~240 GB/s. Inputs are split 5/3 across the sync/scalar rings and
   all issued up front; outputs go per-1024 to gpsimd SWDGE (evens) / the
   by-then-idle sync ring (odds).
 - ewt rides first on the sync ring and the ACT-table warm-up sits right
   after ew on the scalar ring, before the input issues, so the Sqrt table
   (~2.6us) and bias are ready before the first real activation.
 - the PE boots HAM-throttled at half clock; 3 dummy matmuls during the
   DMA-latency window warm it up.
"""

import numpy as np

BANK = 131072
DIM = 64
BATCH = 64
N_CORES = 8
SHARD = BANK // N_CORES  # 16384 bank rows per core
HALF = SHARD // 2  # 8192 columns per partition-half
W = 1024  # chunk width (bank columns per partition-half)
NBLK = 512  # psum bank / matmul block
N_WARM = 4  # PE warm-up dummy matmuls
N_SYNC_IN = 5  # input chunks issued on the sync ring (rest on scalar)

_cache = {}

# test.py reads this after calling kernel() to get profiling info.
last_run = None


def _build(fused_norm=True, half=HALF, w=W):
    import concourse.mybir as mybir
    import concourse.tile as tile
    from concourse import bacc

    f32 = mybir.dt.float32
    bf16 = mybir.dt.bfloat16
    SQRT = mybir.ActivationFunctionType.Sqrt
    nchunk = half // w

    nc = bacc.Bacc(
        "TRN2", target_bir_lowering=False, debug=False, num_devices=N_CORES
    )
    bt = nc.dram_tensor("bt", [128, half], bf16, kind="ExternalInput").ap()
    ew = nc.dram_tensor("ew", [128, 128], f32, kind="ExternalInput").ap()
    ewt = nc.dram_tensor("ewt", [128, DIM], f32, kind="ExternalInput").ap()
    o = nc.dram_tensor("o", [128, half], bf16, kind="ExternalOutput").ap()

    with tile.TileContext(nc) as tc:
        with (
            tc.tile_pool(name="singles", bufs=1) as singles,
            tc.tile_pool(name="bt_pool", bufs=nchunk) as bt_pool,
            tc.tile_pool(name="sq_pool", bufs=3) as sq_pool,
            tc.tile_pool(name="out_pool", bufs=3) as out_pool,
            tc.tile_pool(name="psum", bufs=4, space="PSUM") as psum,
        ):
            # --- input streams, all doorbells rung up front ---------------
            # sync ring: ewt (tiny, feeds bias), then the first chunks.
            # scalar ring: ew (feeds the stationary), ACT warm-up, then the
            # remaining chunks (so table load isn't queued behind 5 DMAs).
            ew2 = singles.tile([128, 128], f32)
            nc.scalar.dma_start(out=ew2, in_=ew)
            ewt2 = singles.tile([128, DIM], f32)
            nc.scalar.dma_start(out=ewt2, in_=ewt)

            # Preload the Sqrt ACT table with the production signature.
            zt = singles.tile([128, 1], f32)
            nc.vector.memset(zt, 0.0)
            warm = singles.tile([128, 1], f32)
            nc.scalar.activation(out=warm, in_=zt, func=SQRT, bias=zt, scale=1.0)

            bts = []
            for pi in range(nchunk // 2):
                bts.append(
                    bt_pool.tile([128, 2 * w], bf16, name=f"bt{pi}", tag="bt")
                )
            for pi in range(nchunk // 2):
                eng = nc.sync if pi < nchunk // 4 else nc.scalar
                eng.dma_start(
                    out=bts[pi], in_=bt[:, pi * 2 * w : (pi + 1) * 2 * w]
                )

            # --- PE warm-up against the HAM clock gate --------------------
            dummy_w = singles.tile([128, 128], bf16)
            nc.gpsimd.memset(dummy_w, 0.0)
            dummy_r = singles.tile([128, 512], bf16)
            nc.gpsimd.memset(dummy_r, 0.0)
            ps_warm = psum.tile([128, w], f32, tag="ps")
            for _ in range(N_WARM):
                nc.tensor.matmul(
                    ps_warm[:, 0:512], lhsT=dummy_w, rhs=dummy_r,
                    start=True, stop=True,
                )

            # Block-diagonal stationary first (gates the first matmul).
            em2bd_f = singles.tile([128, 128], f32)
            nc.vector.memset(em2bd_f, 0.0)
            nc.vector.tensor_scalar_mul(
                em2bd_f[0:64, 0:64], ew2[0:64, 0:DIM], -2.0
            )
            nc.vector.tensor_scalar_mul(
                em2bd_f[64:128, 64:128], ew2[64:128, 0:DIM], -2.0
            )
            em2bd = singles.tile([128, 128], bf16)
            nc.vector.tensor_copy(out=em2bd, in_=em2bd_f)

            # bias[m] = ||e_{m%64}||^2 (+1 for the normalized bank rows),
            # f32 via DVE square + free-dim reduce.
            sq_ewt = singles.tile([128, DIM], f32)
            nc.vector.tensor_mul(sq_ewt, ewt2, ewt2)
            bias = singles.tile([128, 1], f32)
            nc.vector.tensor_reduce(
                out=bias,
                in_=sq_ewt,
                axis=mybir.AxisListType.X,
                op=mybir.AluOpType.add,
            )
            if fused_norm:
                bias1 = singles.tile([128, 1], f32)
                nc.vector.tensor_scalar_add(bias1, bias, 1.0)
                bias = bias1

            if not fused_norm:
                onesbd_f = singles.tile([128, 128], f32)
                nc.vector.memset(onesbd_f, 0.0)
                nc.vector.memset(onesbd_f[0:64, 0:64], 1.0)
                nc.vector.memset(onesbd_f[64:128, 64:128], 1.0)
                onesbd = singles.tile([128, 128], bf16)
                nc.vector.tensor_copy(out=onesbd, in_=onesbd_f)

            # --- main pipeline --------------------------------------------
            for ci in range(nchunk):
                bt_c = bts[ci // 2][:, (ci % 2) * w : (ci % 2 + 1) * w]
                ps = psum.tile([128, w], f32, tag="ps")
                for j in range(w // NBLK):
                    sl = slice(j * NBLK, (j + 1) * NBLK)
                    nc.tensor.matmul(
                        ps[:, sl], lhsT=em2bd, rhs=bt_c[:, sl],
                        start=True, stop=fused_norm,
                    )
                if not fused_norm:
                    sq_c = sq_pool.tile([128, w], bf16)
                    nc.vector.tensor_mul(sq_c, bt_c, bt_c)
                    for j in range(w // NBLK):
                        sl = slice(j * NBLK, (j + 1) * NBLK)
                        nc.tensor.matmul(
                            ps[:, sl], lhsT=onesbd, rhs=sq_c[:, sl],
                            start=False, stop=True,
                        )
                cs = slice(ci * w, (ci + 1) * w)
                out_c = out_pool.tile([128, w], bf16)
                nc.scalar.activation(
                    out=out_c, in_=ps, func=SQRT, bias=bias, scale=1.0
                )
                if ci % 2 == 0:
                    nc.gpsimd.dma_start(out=o[:, cs], in_=out_c)
                else:
                    nc.sync.dma_start(out=o[:, cs], in_=out_c)

    nc.compile()
    return nc


def _get_nc(fused_norm):
    key = ("nc", fused_norm)
    if key not in _cache:
        _cache[key] = _build(fused_norm)
    return _cache[key]


def _prep_inputs(emb_batch, bank):
    """Host-side re-layout only (shard, transpose, stack, bf16 cast)."""
    import ml_dtypes

    bf16 = ml_dtypes.bfloat16
    emb_batch = np.asarray(emb_batch, dtype=np.float32)
    bank = np.asarray(bank, dtype=np.float32)
    # [128, 128]: rows 0-63 and 64-127 both embT; cols duplicated.
    ew_host = np.ascontiguousarray(np.tile(emb_batch.T, (2, 2)))
    # [128, 64] query-major copy for the on-device ||e||^2 reduce.
    ewt_host = np.ascontiguousarray(np.tile(emb_batch, (2, 1)))
    bankT = bank.T  # [64, BANK] view
    in_maps = []
    for c in range(N_CORES):
        sh = bankT[:, c * SHARD : (c + 1) * SHARD]
        btc = np.ascontiguousarray(
            np.concatenate([sh[:, :HALF], sh[:, HALF:]], axis=0)
        ).astype(bf16)
        in_maps.append({"bt": btc, "ew": ew_host, "ewt": ewt_host})
    return in_maps


def kernel(emb_batch, bank):
    global last_run
    from concourse.bass_utils import run_bass_kernel_spmd

    bank = np.asarray(bank, dtype=np.float32)
    # Read-only validation: MemoryBank rows are L2-normalized. Use the
    # fused-constant kernel when that holds, the general one otherwise.
    norms = np.einsum("bd,bd->b", bank, bank)
    fused_norm = bool(np.abs(norms - 1.0).max() < 1e-3)

    nc = _get_nc(fused_norm)
    in_maps = _prep_inputs(emb_batch, bank)
    res = run_bass_kernel_spmd(nc, in_maps, core_ids=list(range(N_CORES)))
    last_run = res
    out = np.empty((BATCH, BANK), dtype=np.float32)
    for c in range(N_CORES):
        oc = res.results[c]["o"]  # [128, HALF] bf16: rows (h*64 + n)
        oc = np.asarray(oc).astype(np.float32)
        out[:, c * SHARD : c * SHARD + HALF] = oc[0:64]
        out[:, c * SHARD + HALF : (c + 1) * SHARD] = oc[64:128]
    return out
